# revision 13
# baseline (speedup 1.0000x reference)
"""CPR linear (int8-dequant matmul with column reordering) on 8 Trainium2
NeuronCores.

Math: y = x[:, col_indices] @ (W_int8 * repeat(scales, gs)) + bias
The column permutation is applied to x on the host (a row gather of the
already-transposed activation matrix), so W stays in natural row order and
the per-group scales stay compact: scale row g applies to W k-rows
[g*128, (g+1)*128) == k-tile g exactly.

Sharding: column-parallel. Each core owns 512 output features: its slice
of W, compact scales, and bias; x is replicated.

Mixed precision: the last NF=8 of 32 k-tiles run as fp8e4 DoubleRow
k-pair matmuls (measured ~2.4x bf16 per k-tile on HW); the other 24 run
bf16. Host pre-quantizes x rows [3072:4096] and W*scales rows [3072:4096]
to e4m3. Quantization raises rel err to ~1.9e-2 (budget 2e-2); the split
count NF is chosen to stay under the gate.

Per-core device kernel:
  - bias broadcast [512] -> [128, 512] via DMA (partition-stride-0)
  - scales broadcast (24 groups) -> [128, 24, 512] via DMA, chunked
  - bf16 dequant: wd[:, kt] = wraw(ints) * sbc (bf16), resident 2.9MB
  - fp8 W: pre-scaled on host, single 256KB DMA, resident [128, 8, 512]
  - main loop over 16 m-blocks of 512 rows:
      6 bf16 x DMAs + 1 fp8 x DMA per block
      block 0: kt-outer with 4 live PSUM banks (W still streaming)
      blocks 1+: per m-subtile: 24 bf16 matmuls + 4 fp8 DoubleRow pairs
      PSUM + bias -> SBUF -> DMA out
"""
from contextlib import ExitStack

import numpy as np
import ml_dtypes

import concourse.bass as bass
import concourse.bacc as bacc
import concourse.mybir as mybir
import concourse.tile as tile

B, S, K, N = 4, 2048, 4096, 4096
M = B * S                    # 8192
NCORES = 8
NS = N // NCORES             # 512 output cols per core
P = 128
NKT = K // P                 # 32 k-tiles
GROUPS = 32
MB = 512                     # m-block rows
NMB = M // MB                # 16
MSUB = MB // P               # 4

bf16 = mybir.dt.bfloat16
f32 = mybir.dt.float32
f8 = mybir.dt.float8e4

NF = 8                       # k-tiles processed as fp8 DoubleRow pairs
KF = NF * P                  # 1024 fp8 k-rows (the last KF rows of x_perm/W)
NKT16 = NKT - NF             # 24 bf16 k-tiles
K16 = NKT16 * P              # 3072 bf16 k-rows

KB = 4                       # k-tiles batched per x-load DMA (512KB transfers)
NKG = NKT16 // KB            # 6 bf16 k-groups


def build(repeats: int = 1, variant: str = "full"):
    """variant: "full" (mixed bf16+fp8) | probe variants (timing only):
    "nomm"/"mmonly"/"mmonly256" (old all-bf16 kernel paths) | "mmxmov"
    (bf16 wd-stationary) | "mmfp8" (all k-tiles fp8 DoubleRow)"""
    if variant == "full":
        return _build_full(repeats)
    if variant == "mmxmov":
        return _build_mmxmov(repeats)
    if variant == "mmfp8":
        return _build_mmfp8(repeats)
    if variant == "mmfp8s":
        return _build_mmfp8(repeats, swinterleave=True)
    return _build_bf16probe(repeats, variant)


def _build_full(repeats: int = 1):
    nc = bacc.Bacc(None)
    # x supplied pre-permuted + pre-transposed (host gather+cast):
    # bf16 rows [0:3072] and e4m3 rows [3072:4096]
    x_d = nc.dram_tensor("xbf", [K16, M], bf16, kind="ExternalInput")
    x8_d = nc.dram_tensor("x8", [KF, M], f8, kind="ExternalInput")
    w_d = nc.dram_tensor("wbf", [K16, NS], bf16, kind="ExternalInput")
    w8_d = nc.dram_tensor("w8", [KF, NS], f8, kind="ExternalInput")
    s_d = nc.dram_tensor("sbf", [NKT16 * NS], bf16, kind="ExternalInput")
    b_d = nc.dram_tensor("bias", [NS], f32, kind="ExternalInput")
    y_d = nc.dram_tensor("y", [M, NS], f32, kind="ExternalOutput")

    DR = mybir.MatmulPerfMode.DoubleRow

    with tile.TileContext(nc) as tc, ExitStack() as stk:
        if repeats > 1:
            stk.enter_context(tc.For_i(0, repeats, 1))
        with (
            tc.tile_pool(name="consts", bufs=1) as consts,
            tc.tile_pool(name="xpool", bufs=3) as xpool,
            tc.tile_pool(name="opool", bufs=2) as opool,
            tc.tile_pool(name="psum", bufs=8, space="PSUM") as psum_pool,
        ):
            # dequantized bf16 weights, resident: [128, 24*NS] bf16.
            # W staged in chunks, smallest first, so the first matmuls gate
            # on only a small load + small dequant. Scales arrive compact
            # and are partition-broadcast by the DMA.
            bias_t = consts.tile([P, NS], f32)
            sbc = consts.tile([P, NKT16, NS], bf16)
            wd = consts.tile([P, NKT16 * NS], bf16)
            wd8 = consts.tile([P, NF, NS], f8)
            with tc.tile_pool(name="wstage", bufs=2) as wstage:
                W_CHUNKS = [1, 1, 2, 4, 8, 8]
                k0 = 0
                for h, H in enumerate(W_CHUNKS):
                    r = slice(k0 * P, (k0 + H) * P)
                    wraw = wstage.tile([P, 8, NS], bf16, tag="wraw")
                    nc.scalar.dma_start(
                        out=wraw[:, :H],
                        in_=w_d[r, :].rearrange("(t p) n -> p t n", p=P))
                    nc.scalar.dma_start(
                        out=sbc[:, k0:k0 + H],
                        in_=bass.AP(tensor=s_d, offset=k0 * NS,
                                    ap=[[0, P], [1, H * NS]]),
                    )
                    nc.vector.tensor_tensor(
                        out=wd[:, k0 * NS:(k0 + H) * NS],
                        in0=wraw[:, :H].opt(), in1=sbc[:, k0:k0 + H].opt(),
                        op=mybir.AluOpType.mult,
                    )
                    k0 += H

            # fp8 W: pre-scaled on host, used only at the tail of block 0's
            # accumulation, so its DMA rides after the bf16 chunks
            nc.scalar.dma_start(
                out=wd8, in_=w8_d.rearrange("(t p) n -> p t n", p=P))
            # bias broadcast to all partitions (needed only at first PSUM
            # eviction, so issued after the W loads on the same queue)
            nc.scalar.dma_start(
                out=bias_t,
                in_=bass.AP(tensor=b_d, offset=0, ap=[[0, P], [1, NS]]),
            )

            def load_block(mb):
                m0 = mb * MB
                xT = []
                for kg in range(NKG):
                    t = xpool.tile([P, KB, MB], bf16, tag=f"xT{kg}")
                    src = x_d[kg * KB * P:(kg + 1) * KB * P, m0:m0 + MB]
                    nc.sync.dma_start(
                        out=t, in_=src.rearrange("(b p) m -> p b m", p=P),
                    )
                    xT.append(t)
                t8 = xpool.tile([P, NF, MB], f8, tag="xT8")
                nc.sync.dma_start(
                    out=t8,
                    in_=x8_d[:, m0:m0 + MB].rearrange(
                        "(t p) m -> p t m", p=P),
                )
                return xT, t8

            # Per block: 4 psum banks, one per m-subtile, each written by
            # one CONTIGUOUS chain (no per-instruction bank cycling, which
            # triggers HAM micro-idle oscillation). The 16 fp8 DoubleRow
            # insts form one contiguous burst so there are only 2 bf16<->
            # fp8 mode transitions per block and the fp8 ldweights
            # pipeline within the burst:
            #   [b16 ms0 x24][f8 ms0..3 x16][b16 ms1 x24][b16 ms2][b16 ms3]
            def mm16(ps, xT, ms, kt, start, stop):
                nc.tensor.matmul(
                    ps,
                    xT[kt // KB][:, kt % KB, ms * P:(ms + 1) * P],
                    wd[:, kt * NS:(kt + 1) * NS],
                    start=start, stop=stop,
                )

            def mm8(ps, t8, ms, kp, start, stop):
                nc.tensor.matmul(
                    ps,
                    t8[:, 2 * kp:2 * kp + 2, ms * P:(ms + 1) * P],
                    wd8[:, 2 * kp:2 * kp + 2, :],
                    start=start, stop=stop,
                    perf_mode=DR,
                )

            # LAYOUT options for the per-block matmul order (empirically
            # chosen; the fp8/bf16 mode-transition and psum-bank-cycling
            # costs are hard to predict):
            #   "v1": per-subtile [b16 x24][f8 x4], subtile-major
            #   "v1r": per-subtile [f8 x4][b16 x24]
            #   "v3": [b16 ms0][f8 ms0..3][b16 ms1..3]
            LAYOUT = "v1"

            for mb in range(NMB):
                m0 = mb * MB
                xT, t8 = load_block(mb)
                pss = []
                for _ in range(MSUB):
                    ps = psum_pool.tile([P, NS], f32, tag="ps")
                    pss.append(ps)
                if LAYOUT == "v1":
                    for ms in range(MSUB):
                        for kt in range(NKT16):
                            mm16(pss[ms], xT, ms, kt,
                                 start=(kt == 0), stop=False)
                        for kp in range(NF // 2):
                            mm8(pss[ms], t8, ms, kp, start=False,
                                stop=(kp == NF // 2 - 1))
                elif LAYOUT == "v1r":
                    for ms in range(MSUB):
                        for kp in range(NF // 2):
                            mm8(pss[ms], t8, ms, kp,
                                start=(kp == 0), stop=False)
                        for kt in range(NKT16):
                            mm16(pss[ms], xT, ms, kt, start=False,
                                 stop=(kt == NKT16 - 1))
                else:  # v3
                    for kt in range(NKT16):
                        mm16(pss[0], xT, 0, kt, start=(kt == 0), stop=False)
                    for kp in range(NF // 2):
                        mm8(pss[0], t8, 0, kp, start=False,
                            stop=(kp == NF // 2 - 1))
                    for ms in range(1, MSUB):
                        for kp in range(NF // 2):
                            mm8(pss[ms], t8, ms, kp,
                                start=(kp == 0), stop=False)
                    for ms in range(1, MSUB):
                        for kt in range(NKT16):
                            mm16(pss[ms], xT, ms, kt, start=False,
                                 stop=(kt == NKT16 - 1))
                for msp in range(MSUB // 2):
                    ot = opool.tile([P, 2, NS], f32, tag="ot")
                    for half in range(2):
                        nc.vector.tensor_tensor(
                            out=ot[:, half], in0=pss[msp * 2 + half],
                            in1=bias_t, op=mybir.AluOpType.add,
                        )
                    row0 = m0 + msp * 2 * P
                    dst = y_d[row0:row0 + 2 * P, :]
                    nc.scalar.dma_start(
                        out=dst.rearrange("(b p) n -> p b n", p=P), in_=ot,
                    )

    nc.compile()
    return nc


# ---------------------------------------------------------------------------
# Timing probe builds (math may be wrong; only instruction timing matters)
# ---------------------------------------------------------------------------

NKG_ALL = NKT // KB          # 8 k-groups covering all 32 k-tiles


def _build_bf16probe(repeats: int = 1, variant: str = "mmonly"):
    """The old all-bf16 kernel: "nomm" (DMA/DVE only) | "mmonly" (PE only)
    | "mmonly256" (PE only, half-width moving operand)."""
    do_mm = variant in ("mmonly", "mmonly256")
    do_xdma = variant == "nomm"
    nw = 256 if variant == "mmonly256" else NS

    nc = bacc.Bacc(None)
    x_d = nc.dram_tensor("xbf", [K, M], bf16, kind="ExternalInput")
    w_d = nc.dram_tensor("wbf", [K, NS], bf16, kind="ExternalInput")
    s_d = nc.dram_tensor("sbf", [GROUPS * NS], bf16, kind="ExternalInput")
    b_d = nc.dram_tensor("bias", [NS], f32, kind="ExternalInput")
    y_d = nc.dram_tensor("y", [M, NS], f32, kind="ExternalOutput")

    with tile.TileContext(nc) as tc, ExitStack() as stk:
        if repeats > 1:
            stk.enter_context(tc.For_i(0, repeats, 1))
        with (
            tc.tile_pool(name="consts", bufs=1) as consts,
            tc.tile_pool(name="xpool", bufs=3) as xpool,
            tc.tile_pool(name="opool", bufs=2) as opool,
            tc.tile_pool(name="psum", bufs=8, space="PSUM") as psum_pool,
        ):
            bias_t = consts.tile([P, NS], f32)
            sbc = consts.tile([P, GROUPS, NS], bf16)
            wd = consts.tile([P, NKT * NS], bf16)
            with tc.tile_pool(name="wstage", bufs=2) as wstage:
                W_CHUNKS = [1, 1, 2, 4, 8, 8, 8]
                k0 = 0
                for h, H in enumerate(W_CHUNKS):
                    r = slice(k0 * P, (k0 + H) * P)
                    wraw = wstage.tile([P, 8, NS], bf16, tag="wraw")
                    nc.scalar.dma_start(
                        out=wraw[:, :H],
                        in_=w_d[r, :].rearrange("(t p) n -> p t n", p=P))
                    nc.scalar.dma_start(
                        out=sbc[:, k0:k0 + H],
                        in_=bass.AP(tensor=s_d, offset=k0 * NS,
                                    ap=[[0, P], [1, H * NS]]),
                    )
                    nc.vector.tensor_tensor(
                        out=wd[:, k0 * NS:(k0 + H) * NS],
                        in0=wraw[:, :H].opt(), in1=sbc[:, k0:k0 + H].opt(),
                        op=mybir.AluOpType.mult,
                    )
                    k0 += H

            nc.scalar.dma_start(
                out=bias_t,
                in_=bass.AP(tensor=b_d, offset=0, ap=[[0, P], [1, NS]]),
            )

            xT_static = None
            if not do_xdma:
                xT_static = []
                for kg in range(NKG_ALL):
                    ts_tile = consts.tile([P, KB, MB], bf16, tag=f"xTs{kg}")
                    nc.vector.memset(ts_tile, 0.5)
                    xT_static.append(ts_tile)

            def load_block(mb):
                m0 = mb * MB
                xT = []
                for kg in range(NKG_ALL):
                    t = xpool.tile([P, KB, MB], bf16, tag=f"xT{kg}")
                    src = x_d[kg * KB * P:(kg + 1) * KB * P, m0:m0 + MB]
                    nc.sync.dma_start(
                        out=t, in_=src.rearrange("(b p) m -> p b m", p=P),
                    )
                    xT.append(t)
                return xT

            mb = 0
            while mb < NMB:
                m0 = mb * MB
                if do_mm and mb == 0:
                    xT0 = load_block(0) if do_xdma else xT_static
                    pss = []
                    for _ in range(MSUB):
                        ps = psum_pool.tile([P, nw], f32, tag="ps")
                        pss.append(ps)
                    for kt in range(NKT):
                        for ms in range(MSUB):
                            nc.tensor.matmul(
                                pss[ms],
                                xT0[kt // KB][:, kt % KB, ms * P:(ms + 1) * P],
                                wd[:, kt * NS:kt * NS + nw],
                                start=(kt == 0), stop=(kt == NKT - 1),
                            )
                    for msp in range(MSUB // 2):
                        ot = opool.tile([P, 2, nw], f32, tag="ot")
                        for half in range(2):
                            nc.vector.tensor_tensor(
                                out=ot[:, half], in0=pss[msp * 2 + half],
                                in1=bias_t[:, :nw], op=mybir.AluOpType.add,
                            )
                        row0 = msp * 2 * P
                        dst = y_d[row0:row0 + 2 * P, :nw]
                        nc.scalar.dma_start(
                            out=dst.rearrange("(b p) n -> p b n", p=P), in_=ot,
                        )
                    mb = 1
                    continue
                if do_xdma:
                    xT = load_block(mb)
                else:
                    xT = xT_static
                if not do_mm:
                    mb += 1
                    continue
                for msp in range(MSUB // 2):
                    ot = opool.tile([P, 2, nw], f32, tag="ot")
                    for half in range(2):
                        ms = msp * 2 + half
                        ps = psum_pool.tile([P, nw], f32, tag="ps")
                        for kt in range(NKT):
                            nc.tensor.matmul(
                                ps,
                                xT[kt // KB][:, kt % KB, ms * P:(ms + 1) * P],
                                wd[:, kt * NS:kt * NS + nw],
                                start=(kt == 0), stop=(kt == NKT - 1),
                            )
                        nc.vector.tensor_tensor(
                            out=ot[:, half], in0=ps, in1=bias_t[:, :nw],
                            op=mybir.AluOpType.add,
                        )
                    row0 = m0 + msp * 2 * P
                    dst = y_d[row0:row0 + 2 * P, :nw]
                    nc.scalar.dma_start(
                        out=dst.rearrange("(b p) n -> p b n", p=P), in_=ot,
                    )
                mb += 1

    nc.compile()
    return nc


def _build_mmfp8(repeats: int = 1, swinterleave: bool = False):
    """Timing probe only (wrong math): all 32 k-tiles processed as 16
    fp8e4 DoubleRow k-pair matmuls per (m-subtile); x/W are static memset
    fp8 tiles. 1024 DoubleRow insts vs 2048 bf16 insts in "mmonly"."""
    nc = bacc.Bacc(None)
    x_d = nc.dram_tensor("xbf", [K, M], bf16, kind="ExternalInput")
    w_d = nc.dram_tensor("wbf", [K, NS], bf16, kind="ExternalInput")
    s_d = nc.dram_tensor("sbf", [GROUPS * NS], bf16, kind="ExternalInput")
    b_d = nc.dram_tensor("bias", [NS], f32, kind="ExternalInput")
    y_d = nc.dram_tensor("y", [M, NS], f32, kind="ExternalOutput")

    with tile.TileContext(nc) as tc, ExitStack() as stk:
        if repeats > 1:
            stk.enter_context(tc.For_i(0, repeats, 1))
        with (
            tc.tile_pool(name="consts", bufs=1) as consts,
            tc.tile_pool(name="opool", bufs=2) as opool,
            tc.tile_pool(name="psum", bufs=8, space="PSUM") as psum_pool,
        ):
            bias_t = consts.tile([P, NS], f32)
            nc.scalar.dma_start(
                out=bias_t,
                in_=bass.AP(tensor=b_d, offset=0, ap=[[0, P], [1, NS]]),
            )
            wd8 = consts.tile([P, NKT, NS], f8)
            nc.vector.memset(wd8, 0.25)
            xT8_static = []
            for kg in range(NKG_ALL):
                ts_tile = consts.tile([P, KB, MB], f8, tag=f"xT8s{kg}")
                nc.vector.memset(ts_tile, 0.5)
                xT8_static.append(ts_tile)

            for mb in range(NMB):
                m0 = mb * MB
                for msp in range(MSUB // 2):
                    ot = opool.tile([P, 2, NS], f32, tag="ot")
                    for half in range(2):
                        ms = msp * 2 + half
                        ps = psum_pool.tile([P, NS], f32, tag="ps")
                        pm = (mybir.MatmulPerfMode.DoubleRowSwInterleave
                              if swinterleave
                              else mybir.MatmulPerfMode.DoubleRow)
                        for kp in range(NKT // 2):
                            kt = 2 * kp
                            if swinterleave:
                                # SW-interleaved stationary: contiguous
                                # [2*P] free elements per k-pair (probe:
                                # any 256 contiguous elements)
                                o = (ms % 2) * 2 * P
                                lhsT = xT8_static[kt // KB][
                                    :, kt % KB, o:o + 2 * P]
                            else:
                                lhsT = xT8_static[kt // KB][
                                    :, kt % KB:kt % KB + 2,
                                    ms * P:(ms + 1) * P]
                            nc.tensor.matmul(
                                ps,
                                lhsT,
                                wd8[:, kt:kt + 2, :],
                                start=(kp == 0), stop=(kp == NKT // 2 - 1),
                                perf_mode=pm,
                            )
                        nc.vector.tensor_tensor(
                            out=ot[:, half], in0=ps, in1=bias_t,
                            op=mybir.AluOpType.add,
                        )
                    row0 = m0 + msp * 2 * P
                    dst = y_d[row0:row0 + 2 * P, :]
                    nc.scalar.dma_start(
                        out=dst.rearrange("(b p) n -> p b n", p=P), in_=ot,
                    )

    nc.compile()
    return nc


def _build_mmxmov(repeats: int = 1):
    """Timing probe only (wrong math): wd is the stationary operand, x the
    moving one; each ldweights feeds 2 matmuls of 512 moving columns."""
    nc = bacc.Bacc(None)
    x_d = nc.dram_tensor("xbf", [K, M], bf16, kind="ExternalInput")
    w_d = nc.dram_tensor("wbf", [K, NS], bf16, kind="ExternalInput")
    s_d = nc.dram_tensor("sbf", [GROUPS * NS], bf16, kind="ExternalInput")
    b_d = nc.dram_tensor("bias", [NS], f32, kind="ExternalInput")
    y_d = nc.dram_tensor("y", [M, NS], f32, kind="ExternalOutput")

    with tile.TileContext(nc) as tc, ExitStack() as stk:
        if repeats > 1:
            stk.enter_context(tc.For_i(0, repeats, 1))
        with (
            tc.tile_pool(name="consts", bufs=1) as consts,
            tc.tile_pool(name="opool", bufs=2) as opool,
            tc.tile_pool(name="psum", bufs=8, space="PSUM") as psum_pool,
        ):
            bias_t = consts.tile([P, NS], f32)
            nc.scalar.dma_start(
                out=bias_t,
                in_=bass.AP(tensor=b_d, offset=0, ap=[[0, P], [1, NS]]),
            )
            wd = consts.tile([P, NKT * NS], bf16)
            nc.vector.memset(wd, 0.25)
            xT_static = []
            for kg in range(NKG_ALL):
                ts_tile = consts.tile([P, KB, MB], bf16, tag=f"xTs{kg}")
                nc.vector.memset(ts_tile, 0.5)
                xT_static.append(ts_tile)

            NT = NS // P                             # 4 n-tiles
            for mp in range(NMB // 2):               # block pairs: 1024 m rows
                pss = []
                for _ in range(2 * NT):
                    ps = psum_pool.tile([P, MB], f32, tag="ps")
                    pss.append(ps)
                for kt in range(NKT):
                    xt = xT_static[kt // KB][:, kt % KB]   # [128k, 512m]
                    for nt in range(NT):
                        wslice = wd[:, kt * NS + nt * P:kt * NS + (nt + 1) * P]
                        for half in range(2):
                            nc.tensor.matmul(
                                pss[nt * 2 + half], wslice, xt,
                                start=(kt == 0), stop=(kt == NKT - 1),
                            )
                for nt in range(NT):
                    ot = opool.tile([P, 2, MB], f32, tag="ot")
                    for half in range(2):
                        nc.vector.tensor_tensor(
                            out=ot[:, half], in0=pss[nt * 2 + half],
                            in1=bias_t[:, :MB],
                            op=mybir.AluOpType.add,
                        )
                    row0 = (mp * 2 * NT + nt * 2) * P
                    dst = y_d[row0:row0 + 2 * P, :]
                    nc.scalar.dma_start(
                        out=dst.rearrange("(b p) n -> p b n", p=P),
                        in_=ot[:, :, :NS],
                    )

    nc.compile()
    return nc


def make_in_maps(x, scales, bias, weight_int8, col_indices, group_size):
    """Host-side sharding/layout prep: index gathers, dtype casts, and the
    fp8 pre-quantization of the last NF k-tiles."""
    x2 = np.asarray(x, dtype=np.float32).reshape(M, K)
    ci = np.asarray(col_indices).astype(np.int64)
    # permutation applied to x (rows of x^T); W stays in natural row order
    xT = np.ascontiguousarray(x2.T)[ci]                         # [K, M] f32
    x_bf = xT[:K16].astype(ml_dtypes.bfloat16)                  # [K16, M]
    x_f8 = xT[K16:].astype(ml_dtypes.float8_e4m3)               # [KF, M]

    Wn = np.asarray(weight_int8)             # [K, N], int32 values in [-128,127]
    sc = np.asarray(scales, dtype=np.float32)
    bias = np.asarray(bias, dtype=np.float32)
    s_full_f = np.repeat(sc[NKT16:], int(group_size), axis=0)   # [KF, N]
    w8_full = (Wn[K16:].astype(np.float32) * s_full_f).astype(
        ml_dtypes.float8_e4m3)                                  # [KF, N]

    in_maps = []
    for c in range(NCORES):
        cols = slice(c * NS, (c + 1) * NS)
        in_maps.append({
            "xbf": x_bf,
            "x8": x_f8,
            "wbf": Wn[:K16, cols].astype(ml_dtypes.bfloat16),   # exact (ints)
            "w8": np.ascontiguousarray(w8_full[:, cols]),
            "sbf": np.ascontiguousarray(
                sc[:NKT16, cols].astype(ml_dtypes.bfloat16)).reshape(-1),
            "bias": bias[cols],
        })
    return in_maps


REPL_NAMES = ("xbf", "x8")

_RUNNER = None


def _make_runner():
    """Build the bass module once and wrap it in a cached sharded jit."""
    import jax
    from jax.sharding import Mesh, PartitionSpec, NamedSharding
    from jax.experimental.shard_map import shard_map
    from concourse import bass2jax
    from concourse.bass2jax import _bass_exec_p, install_neuronx_cc_hook

    nc = build(repeats=1)
    install_neuronx_cc_hook()
    partition_name = nc.partition_id_tensor.name if nc.partition_id_tensor else None

    in_names, out_names, out_avals, zero_outs = [], [], [], []
    for alloc in nc.m.functions[0].allocations:
        if not isinstance(alloc, mybir.MemoryLocationSet):
            continue
        name = alloc.memorylocations[0].name
        if alloc.kind == "ExternalInput":
            if name != partition_name:
                in_names.append(name)
        elif alloc.kind == "ExternalOutput":
            out_names.append(name)
            shape = tuple(alloc.tensor_shape)
            dtype = mybir.dt.np(alloc.dtype)
            out_avals.append(jax.core.ShapedArray(shape, dtype))
            zero_outs.append(np.zeros(shape, dtype))
    all_in_names = list(in_names) + list(out_names)
    if partition_name is not None:
        all_in_names.append(partition_name)
    n_params, n_outs = len(in_names), len(out_names)

    def _body(*args):
        operands = list(args)
        if partition_name is not None:
            operands.append(bass2jax.partition_id_tensor())
        outs = _bass_exec_p.bind(
            *operands,
            out_avals=tuple(out_avals),
            in_names=tuple(all_in_names),
            out_names=tuple(out_names),
            lowering_input_output_aliases=(),
            sim_require_finite=True,
            sim_require_nnan=True,
            nc=nc,
        )
        return tuple(outs)

    devices = jax.devices()[:NCORES]
    mesh = Mesh(np.asarray(devices), ("core",))
    # x ("xbf"/"x8") is identical on every core: pass it replicated so only
    # one copy crosses the host->device link; per-core tensors are
    # concat-sharded.
    in_specs = tuple(
        PartitionSpec() if name in REPL_NAMES else PartitionSpec("core")
        for name in in_names
    ) + (PartitionSpec("core"),) * n_outs
    sharded = jax.jit(
        shard_map(
            _body, mesh=mesh,
            in_specs=in_specs,
            out_specs=(PartitionSpec("core"),) * n_outs,
            check_rep=False,
        ),
        keep_unused=True,
    )
    shard_core = NamedSharding(mesh, PartitionSpec("core"))
    shard_repl = NamedSharding(mesh, PartitionSpec())

    def run(in_maps):
        import jax as _jax
        dev_in = []
        for name in in_names:
            if name in REPL_NAMES:
                dev_in.append(
                    _jax.device_put(np.asarray(in_maps[0][name]), shard_repl))
            else:
                a = np.concatenate(
                    [np.asarray(in_maps[c][name]) for c in range(NCORES)], axis=0)
                dev_in.append(_jax.device_put(a, shard_core))
        dev_zero = [
            _jax.device_put(
                np.zeros((NCORES * z.shape[0], *z.shape[1:]), z.dtype), shard_core)
            for z in zero_outs
        ]
        out = sharded(*dev_in, *dev_zero)
        return [
            {name: np.asarray(out[i]).reshape(NCORES, *zero_outs[i].shape)[c]
             for i, name in enumerate(out_names)}
            for c in range(NCORES)
        ]

    return run


def kernel(x, scales, bias, weight_int8, col_indices, group_size):
    global _RUNNER
    in_maps = make_in_maps(x, scales, bias, weight_int8, col_indices, group_size)
    if _RUNNER is None:
        _RUNNER = _make_runner()
    results = _RUNNER(in_maps)
    y = np.concatenate([results[c]["y"] for c in range(NCORES)], axis=1)
    return np.ascontiguousarray(y.reshape(B, S, N))
